# revision 7
# baseline (speedup 1.0000x reference)
"""Causal single-head attention kernel for TRN2 (one batch element per core).

Computes: out = softmax(causal((X_q Wq + bq)(X_k Wk + bk)^T / sqrt(H))) (X_v Wv + bv)
Shapes per core: Q,K,V [S, E]; Wq/Wk/Wv [E, H]; bq/bk/bv [H]; out [S, H].

v3 design:
- Input X transposes on the PE (bf16, via identity) hidden under the
  HBM-bound streaming phase; PSUM->SBUF copies split scalar/vector.
- DMA issue order: weights/biases first, then all 12 input-chunk DMAs in
  consumption order (Q3 before K3/V3), then constants -- so the first
  projection starts ~7us in and weights never queue behind bulk input.
- One interleaved loop: project K/V/Q of chunk c, then attention chunk c;
  next chunk's DMA + transposes overlap attention. Chunk 3 is split so its
  off-diagonal attention tiles run before K3/V3 are projected.
- Softmax denominators off the PE: gpsimd partition-reduce (axis=C) of each
  exp'd score tile, accumulated on vector; only a tiny [1,128]x[1,1]
  transpose-matmul per chunk stays on the PE.
- V tiles transposed via the DMA XBAR on the idle Sync queue (16 calls);
  output stores issued on gpsimd to keep Sync/Scalar queues unblocked.
"""

from contextlib import ExitStack

import numpy as np

import concourse.bacc as bacc
import concourse.bass as bass
import concourse.mybir as mybir
import concourse.tile as tile
from concourse.masks import make_identity

F32 = mybir.dt.float32
BF16 = mybir.dt.bfloat16

CH = 512          # Sq chunk width (psum bank)
PT = 128          # partition tile


def build(S=2048, E=1024, H=128, n_cores=8):
    """Build + compile the Bacc kernel. Returns nc."""
    EC = E // PT              # E chunks (8)
    NCHUNK = S // CH          # Sq chunks (4)
    TPC = CH // PT            # S-tiles per chunk (4)
    scale = float(H) ** -0.5

    nc = bacc.Bacc("TRN2", target_bir_lowering=False, debug=False,
                   num_devices=n_cores)

    Qd = nc.declare_dram_parameter("Q", [S, E], F32, isOutput=False)
    Kd = nc.declare_dram_parameter("K", [S, E], F32, isOutput=False)
    Vd = nc.declare_dram_parameter("V", [S, E], F32, isOutput=False)
    Wqd = nc.declare_dram_parameter("Wq", [E, H], F32, isOutput=False)
    Wkd = nc.declare_dram_parameter("Wk", [E, H], F32, isOutput=False)
    Wvd = nc.declare_dram_parameter("Wv", [E, H], F32, isOutput=False)
    bqd = nc.declare_dram_parameter("bq", [H], F32, isOutput=False)
    bkd = nc.declare_dram_parameter("bk", [H], F32, isOutput=False)
    bvd = nc.declare_dram_parameter("bv", [H], F32, isOutput=False)
    outd = nc.declare_dram_parameter("out", [S, H], F32, isOutput=True)

    xd = {"q": Qd, "k": Kd, "v": Vd}

    with tile.TileContext(nc) as tc, ExitStack() as ctx:
        persist = ctx.enter_context(tc.tile_pool(name="persist", bufs=1))
        xn_p = ctx.enter_context(tc.tile_pool(name="xn", bufs=1))
        xt_p = ctx.enter_context(tc.tile_pool(name="xt", bufs=12))
        ew_p = ctx.enter_context(tc.tile_pool(name="ew", bufs=8))
        small_p = ctx.enter_context(tc.tile_pool(name="small", bufs=6))

        ps_tp = ctx.enter_context(tc.tile_pool(name="ps_tp", bufs=2, space="PSUM"))
        ps_mm = ctx.enter_context(tc.tile_pool(name="ps_mm", bufs=3, space="PSUM"))
        ps_outT = ctx.enter_context(tc.tile_pool(name="ps_outT", bufs=1, space="PSUM"))
        ps_fin = ctx.enter_context(tc.tile_pool(name="ps_fin", bufs=1, space="PSUM"))

        # ---- weights + biases first (small, land before first projection) ----
        wts = {}
        for nm, d in (("k", Wkd), ("v", Wvd), ("q", Wqd)):
            w = persist.tile([PT, EC, H], BF16, tag=f"w{nm}")
            nc.gpsimd.dma_start(out=w, in_=d[:].rearrange("(c p) h -> p c h", p=PT))
            wts[nm] = w
        bias = {}
        for nm, d in (("k", bkd), ("v", bvd), ("q", bqd)):
            b = persist.tile([H, 1], F32, tag=f"b{nm}")
            nc.gpsimd.dma_start(out=b, in_=d[:].unsqueeze(1))
            bias[nm] = b

        # ---- input stream: all 12 chunk DMAs issued up front (gpsimd casts) ----
        stream = [("k", 0), ("v", 0), ("q", 0),
                  ("k", 1), ("v", 1), ("q", 1),
                  ("k", 2), ("v", 2), ("q", 2),
                  ("q", 3), ("k", 3), ("v", 3)]
        xn = {}
        for nm, c in stream:
            t_ = xn_p.tile([PT, TPC, E], BF16, tag=f"xn_{nm}{c}",
                           name=f"xn_{nm}{c}")
            nc.gpsimd.dma_start(
                out=t_, in_=xd[nm][c * CH:(c + 1) * CH, :].rearrange(
                    "(t p) e -> p t e", p=PT))
            xn[(nm, c)] = t_

        # ---- constants (behind the DMA issues on gpsimd) ----
        ident = persist.tile([PT, PT], F32, tag="ident")
        make_identity(nc, ident)
        ident_b = persist.tile([PT, PT], BF16, tag="ident_b")
        make_identity(nc, ident_b)
        one_1 = persist.tile([1, 1], F32, tag="one_1")
        nc.gpsimd.memset(one_1, 1.0)

        masks = []
        for m in range(TPC):
            mk = persist.tile([PT, CH], BF16, tag=f"mask{m}")
            nc.gpsimd.memset(mk, 1.0)
            # keep (=1.0) where f - p - 128*m >= 0 else fill 0.0
            nc.gpsimd.affine_select(
                out=mk, in_=mk, compare_op=mybir.AluOpType.is_ge,
                fill=0.0, base=-PT * m, pattern=[[1, CH]], channel_multiplier=-1,
            )
            masks.append(mk)

        # persistent projected tensors
        qT = [persist.tile([H, CH], BF16, tag=f"qT{c}", name=f"qT{c}")
              for c in range(NCHUNK)]
        kT = [persist.tile([H, CH], BF16, tag=f"kT{c}", name=f"kT{c}")
              for c in range(NCHUNK)]
        vnat = [persist.tile([PT, H], BF16, tag=f"v{j}", name=f"v{j}")
                for j in range(S // PT)]

        # ---- helpers ----
        def project(nm, c):
            """xn[(nm,c)] -> qT[c]/kT[c]/vnat[4c..4c+3]: PE transpose + PE mm."""
            xnt = xn[(nm, c)]
            w = wts[nm]
            b = bias[nm]
            xts = []
            for e in range(EC):
                tp = ps_tp.tile([PT, CH], BF16, tag="tp")
                for t in range(TPC):
                    nc.tensor.transpose(
                        out=tp[:, t * PT:(t + 1) * PT],
                        in_=xnt[:, t, e * PT:(e + 1) * PT],
                        identity=ident_b[:],
                    )
                xt = xt_p.tile([PT, CH], BF16, tag="xt")
                if e % 2 == 0:
                    nc.scalar.copy(out=xt, in_=tp)
                else:
                    nc.vector.tensor_copy(xt, tp)
                xts.append(xt)
            pj = ps_mm.tile([H, CH], F32, tag="mm")
            for e in range(EC):
                nc.tensor.matmul(pj, w[:, e, :], xts[e],
                                 start=(e == 0), stop=(e == EC - 1))
            if nm == "q":
                nc.vector.tensor_scalar_add(qT[c], pj, b[:])
            elif nm == "k":
                nc.vector.tensor_scalar_add(kT[c], pj, b[:])
            else:
                vTb = small_p.tile([H, CH], BF16, tag="vTb")
                nc.vector.tensor_scalar_add(vTb, pj, b[:])
                for t in range(TPC):
                    nc.sync.dma_start_transpose(
                        out=vnat[c * TPC + t],
                        in_=vTb[:, t * PT:(t + 1) * PT])

        def attn_tiles(c, js, oT, acc):
            """Score/exp/PV for Sk tiles `js` of Sq chunk c.

            acc [1, CH] f32 accumulates softmax denominators (gpsimd
            C-reduce per tile + vector add)."""
            nk = (c + 1) * TPC
            for j in js:
                wp = ps_mm.tile([PT, CH], F32, tag="mm")
                kc, kt = divmod(j, TPC)
                nc.tensor.matmul(wp, kT[kc][:, kt * PT:(kt + 1) * PT],
                                 qT[c], start=True, stop=True)
                ew = ew_p.tile([PT, CH], BF16, tag="ew")
                nc.scalar.activation(out=ew, in_=wp,
                                     func=mybir.ActivationFunctionType.Exp,
                                     scale=scale)
                m = j - c * TPC
                if m >= 0:
                    nc.vector.tensor_mul(ew, ew, masks[m])
                # PV accumulate on PE
                nc.tensor.matmul(oT, vnat[j][:], ew,
                                 start=(j == 0), stop=(j == nk - 1))
                # denominators: partition-reduce on gpsimd, accumulate on vector
                if j == 0:
                    nc.gpsimd.tensor_reduce(
                        out=acc, in_=ew, axis=mybir.AxisListType.C,
                        op=mybir.AluOpType.add)
                else:
                    srow = small_p.tile([1, CH], F32, tag="srow")
                    nc.gpsimd.tensor_reduce(
                        out=srow, in_=ew, axis=mybir.AxisListType.C,
                        op=mybir.AluOpType.add)
                    nc.vector.tensor_add(acc, acc, srow)

        def finalize(c, oT, acc):
            # acc [1, CH] -> sumsT [128, TPC] (tiny PE matmuls), then recip
            sumsT = ps_fin.tile([PT, TPC], F32, tag="sumsT")
            for t in range(TPC):
                nc.tensor.matmul(sumsT[:, t:t + 1],
                                 acc[0:1, t * PT:(t + 1) * PT],
                                 one_1[:], start=True, stop=True)
            recip = small_p.tile([PT, TPC], F32, tag="recip")
            nc.vector.reciprocal(recip, sumsT[:, 0:TPC])

            oT_sb = small_p.tile([H, CH], F32, tag="oT_sb")
            nc.scalar.copy(out=oT_sb, in_=oT)
            otp = ps_fin.tile([PT, CH], F32, tag="otp")
            for t in range(TPC):
                nc.tensor.transpose(out=otp[:, t * PT:(t + 1) * PT],
                                    in_=oT_sb[:, t * PT:(t + 1) * PT],
                                    identity=ident[:])
            for t in range(TPC):
                ob = small_p.tile([PT, H], F32, tag="ob")
                nc.vector.tensor_scalar_mul(ob, otp[:, t * PT:(t + 1) * PT],
                                            recip[:, t:t + 1])
                nc.gpsimd.dma_start(
                    out=outd[c * CH + t * PT: c * CH + (t + 1) * PT, :], in_=ob)

        # ---- interleaved main loop ----
        for c in range(3):
            project("k", c)
            project("v", c)
            project("q", c)
            oT = ps_outT.tile([H, CH], F32, tag="outT")
            acc = small_p.tile([1, CH], F32, tag="acc")
            attn_tiles(c, range((c + 1) * TPC), oT, acc)
            finalize(c, oT, acc)

        # chunk 3: Q first, off-diagonal attention, then K3/V3, diagonal tiles
        project("q", 3)
        oT = ps_outT.tile([H, CH], F32, tag="outT")
        acc = small_p.tile([1, CH], F32, tag="acc")
        attn_tiles(3, range(12), oT, acc)
        project("k", 3)
        project("v", 3)
        attn_tiles(3, range(12, 16), oT, acc)
        finalize(3, oT, acc)

    nc.compile()
    return nc


_NC_CACHE = {}


def _get_nc():
    if "nc" not in _NC_CACHE:
        _NC_CACHE["nc"] = build(S=2048, E=1024, H=128, n_cores=8)
    return _NC_CACHE["nc"]


def kernel(Q, K, V, mask=None, Wq=None, bq=None, Wk=None, bk=None,
           Wv=None, bv=None, **_):
    """Full-input entry point: Q/K/V [8, 2048, 1024] fp32 -> out [8, 2048, 128].

    Data-parallel over batch: core i computes batch element i. The causal
    mask input is ignored (causality is hardcoded in the kernel structure).
    """
    from concourse.bass_utils import run_bass_kernel_spmd

    B = Q.shape[0]
    nc = _get_nc()
    f32 = np.float32
    in_maps = []
    for i in range(B):
        in_maps.append({
            "Q": np.ascontiguousarray(Q[i], dtype=f32),
            "K": np.ascontiguousarray(K[i], dtype=f32),
            "V": np.ascontiguousarray(V[i], dtype=f32),
            "Wq": np.ascontiguousarray(Wq, dtype=f32),
            "Wk": np.ascontiguousarray(Wk, dtype=f32),
            "Wv": np.ascontiguousarray(Wv, dtype=f32),
            "bq": np.ascontiguousarray(bq, dtype=f32),
            "bk": np.ascontiguousarray(bk, dtype=f32),
            "bv": np.ascontiguousarray(bv, dtype=f32),
        })
    r = run_bass_kernel_spmd(nc, in_maps, list(range(B)))
    return np.stack([r.results[i]["out"] for i in range(B)]).astype(np.float32)


# revision 9
# speedup vs baseline: 14.9763x; 14.9763x over previous
"""Causal single-head attention kernel for TRN2 (one batch element per core).

Computes: out = softmax(causal((X_q Wq + bq)(X_k Wk + bk)^T / sqrt(H))) (X_v Wv + bv)
Shapes per core: Q,K,V [S, E]; Wq/Wk/Wv [E, H]; bq/bk/bv [H]; out [S, H].

v3 design:
- Input X transposes on the PE (bf16, via identity) hidden under the
  HBM-bound streaming phase; PSUM->SBUF copies split scalar/vector.
- DMA issue order: weights/biases first, then all 12 input-chunk DMAs in
  consumption order (Q3 before K3/V3), then constants -- so the first
  projection starts ~7us in and weights never queue behind bulk input.
- One interleaved loop: project K/V/Q of chunk c, then attention chunk c;
  next chunk's DMA + transposes overlap attention. Chunk 3 is split so its
  off-diagonal attention tiles run before K3/V3 are projected.
- Softmax denominators off the PE: gpsimd partition-reduce (axis=C) of each
  exp'd score tile, accumulated on vector; only a tiny [1,128]x[1,1]
  transpose-matmul per chunk stays on the PE.
- V tiles transposed via the DMA XBAR on the idle Sync queue (16 calls);
  output stores issued on gpsimd to keep Sync/Scalar queues unblocked.
"""

from contextlib import ExitStack

import numpy as np

import concourse.bacc as bacc
import concourse.bass as bass
import concourse.mybir as mybir
import concourse.tile as tile
from concourse.masks import make_identity

F32 = mybir.dt.float32
BF16 = mybir.dt.bfloat16

CH = 512          # Sq chunk width (psum bank)
PT = 128          # partition tile


def build(S=2048, E=1024, H=128, n_cores=8):
    """Build + compile the Bacc kernel. Returns nc."""
    EC = E // PT              # E chunks (8)
    NCHUNK = S // CH          # Sq chunks (4)
    TPC = CH // PT            # S-tiles per chunk (4)
    scale = float(H) ** -0.5

    nc = bacc.Bacc("TRN2", target_bir_lowering=False, debug=False,
                   num_devices=n_cores)

    Qd = nc.declare_dram_parameter("Q", [S, E], F32, isOutput=False)
    Kd = nc.declare_dram_parameter("K", [S, E], F32, isOutput=False)
    Vd = nc.declare_dram_parameter("V", [S, E], F32, isOutput=False)
    Wqd = nc.declare_dram_parameter("Wq", [E, H], F32, isOutput=False)
    Wkd = nc.declare_dram_parameter("Wk", [E, H], F32, isOutput=False)
    Wvd = nc.declare_dram_parameter("Wv", [E, H], F32, isOutput=False)
    bqd = nc.declare_dram_parameter("bq", [H], F32, isOutput=False)
    bkd = nc.declare_dram_parameter("bk", [H], F32, isOutput=False)
    bvd = nc.declare_dram_parameter("bv", [H], F32, isOutput=False)
    outd = nc.declare_dram_parameter("out", [S, H], F32, isOutput=True)

    xd = {"q": Qd, "k": Kd, "v": Vd}

    with tile.TileContext(nc) as tc, ExitStack() as ctx:
        persist = ctx.enter_context(tc.tile_pool(name="persist", bufs=1))
        xn_p = ctx.enter_context(tc.tile_pool(name="xn", bufs=1))
        xt_p = ctx.enter_context(tc.tile_pool(name="xt", bufs=12))
        ew_p = ctx.enter_context(tc.tile_pool(name="ew", bufs=8))
        small_p = ctx.enter_context(tc.tile_pool(name="small", bufs=6))

        ps_tp = ctx.enter_context(tc.tile_pool(name="ps_tp", bufs=2, space="PSUM"))
        ps_mm = ctx.enter_context(tc.tile_pool(name="ps_mm", bufs=3, space="PSUM"))
        ps_outT = ctx.enter_context(tc.tile_pool(name="ps_outT", bufs=1, space="PSUM"))
        ps_fin = ctx.enter_context(tc.tile_pool(name="ps_fin", bufs=1, space="PSUM"))
        ps_sums = ctx.enter_context(tc.tile_pool(name="ps_sums", bufs=1, space="PSUM"))

        # ---- weights + biases first (small, land before first projection) ----
        wts = {}
        for nm, d in (("k", Wkd), ("v", Wvd), ("q", Wqd)):
            w = persist.tile([PT, EC, H], BF16, tag=f"w{nm}")
            nc.gpsimd.dma_start(out=w, in_=d[:].rearrange("(c p) h -> p c h", p=PT))
            wts[nm] = w
        bias = {}
        for nm, d in (("k", bkd), ("v", bvd), ("q", bqd)):
            b = persist.tile([H, 1], F32, tag=f"b{nm}")
            nc.gpsimd.dma_start(out=b, in_=d[:].unsqueeze(1))
            bias[nm] = b

        # ---- input stream: all 12 chunk DMAs issued up front (gpsimd casts) ----
        stream = [("k", 0), ("v", 0), ("q", 0),
                  ("k", 1), ("v", 1), ("q", 1),
                  ("k", 2), ("v", 2), ("q", 2),
                  ("q", 3), ("k", 3), ("v", 3)]
        xn = {}
        for nm, c in stream:
            t_ = xn_p.tile([PT, TPC, E], BF16, tag=f"xn_{nm}{c}",
                           name=f"xn_{nm}{c}")
            nc.gpsimd.dma_start(
                out=t_, in_=xd[nm][c * CH:(c + 1) * CH, :].rearrange(
                    "(t p) e -> p t e", p=PT))
            xn[(nm, c)] = t_

        # ---- constants (behind the DMA issues on gpsimd) ----
        ident = persist.tile([PT, PT], F32, tag="ident")
        make_identity(nc, ident)
        ident_b = persist.tile([PT, PT], BF16, tag="ident_b")
        make_identity(nc, ident_b)
        one_1 = persist.tile([1, 1], F32, tag="one_1")
        nc.gpsimd.memset(one_1, 1.0)
        ones_col = persist.tile([PT, 1], BF16, tag="ones_col")
        nc.gpsimd.memset(ones_col, 1.0)

        masks = []
        for m in range(TPC):
            mk = persist.tile([PT, CH], BF16, tag=f"mask{m}")
            nc.gpsimd.memset(mk, 1.0)
            # keep (=1.0) where f - p - 128*m >= 0 else fill 0.0
            nc.gpsimd.affine_select(
                out=mk, in_=mk, compare_op=mybir.AluOpType.is_ge,
                fill=0.0, base=-PT * m, pattern=[[1, CH]], channel_multiplier=-1,
            )
            masks.append(mk)

        # persistent projected tensors
        qT = [persist.tile([H, CH], BF16, tag=f"qT{c}", name=f"qT{c}")
              for c in range(NCHUNK)]
        kT = [persist.tile([H, CH], BF16, tag=f"kT{c}", name=f"kT{c}")
              for c in range(NCHUNK)]
        vnat = [persist.tile([PT, H], BF16, tag=f"v{j}", name=f"v{j}")
                for j in range(S // PT)]

        # ---- helpers ----
        def project(nm, c):
            """xn[(nm,c)] -> qT[c]/kT[c]/vnat[4c..4c+3]: PE transpose + PE mm."""
            xnt = xn[(nm, c)]
            w = wts[nm]
            b = bias[nm]
            xts = []
            for e in range(EC):
                tp = ps_tp.tile([PT, CH], BF16, tag="tp")
                for t in range(TPC):
                    nc.tensor.transpose(
                        out=tp[:, t * PT:(t + 1) * PT],
                        in_=xnt[:, t, e * PT:(e + 1) * PT],
                        identity=ident_b[:],
                    )
                xt = xt_p.tile([PT, CH], BF16, tag="xt")
                if e % 2 == 0:
                    nc.scalar.copy(out=xt, in_=tp)
                else:
                    nc.vector.tensor_copy(xt, tp)
                xts.append(xt)
            pj = ps_mm.tile([H, CH], F32, tag="mm")
            for e in range(EC):
                nc.tensor.matmul(pj, w[:, e, :], xts[e],
                                 start=(e == 0), stop=(e == EC - 1))
            if nm == "q":
                nc.vector.tensor_scalar_add(qT[c], pj, b[:])
            elif nm == "k":
                nc.vector.tensor_scalar_add(kT[c], pj, b[:])
            else:
                vTb = small_p.tile([H, CH], BF16, tag="vTb")
                nc.vector.tensor_scalar_add(vTb, pj, b[:])
                for t in range(TPC):
                    nc.sync.dma_start_transpose(
                        out=vnat[c * TPC + t],
                        in_=vTb[:, t * PT:(t + 1) * PT])

        def attn_tiles(c, js, oT, sums):
            """Score/exp/PV for Sk tiles `js` of Sq chunk c.

            sums [1, CH] f32 PSUM accumulates softmax denominators via
            ones-vector matmuls."""
            nk = (c + 1) * TPC
            for j in js:
                wp = ps_mm.tile([PT, CH], F32, tag="mm")
                kc, kt = divmod(j, TPC)
                nc.tensor.matmul(wp, kT[kc][:, kt * PT:(kt + 1) * PT],
                                 qT[c], start=True, stop=True)
                ew = ew_p.tile([PT, CH], BF16, tag="ew")
                nc.scalar.activation(out=ew, in_=wp,
                                     func=mybir.ActivationFunctionType.Exp,
                                     scale=scale)
                m = j - c * TPC
                if m >= 0:
                    nc.vector.tensor_mul(ew, ew, masks[m])
                # PV accumulate on PE
                nc.tensor.matmul(oT, vnat[j][:], ew,
                                 start=(j == 0), stop=(j == nk - 1))
                # denominators on PE: ones^T @ ew accumulated in PSUM
                nc.tensor.matmul(sums, ones_col[:], ew,
                                 start=(j == 0), stop=(j == nk - 1))

        def finalize(c, oT, sums):
            # sums [1, CH] -> sumsT [128, TPC] (tiny PE matmuls), then recip
            sums_sb = small_p.tile([1, CH], F32, tag="sums_sb")
            nc.vector.tensor_copy(sums_sb, sums)
            sumsT = ps_mm.tile([PT, TPC], F32, tag="mm")
            for t in range(TPC):
                nc.tensor.matmul(sumsT[:, t:t + 1],
                                 sums_sb[0:1, t * PT:(t + 1) * PT],
                                 one_1[:], start=True, stop=True)
            recip = small_p.tile([PT, TPC], F32, tag="recip")
            nc.vector.reciprocal(recip, sumsT[:, 0:TPC])

            oT_sb = small_p.tile([H, CH], F32, tag="oT_sb")
            nc.scalar.copy(out=oT_sb, in_=oT)
            otp = ps_fin.tile([PT, CH], F32, tag="otp")
            for t in range(TPC):
                nc.tensor.transpose(out=otp[:, t * PT:(t + 1) * PT],
                                    in_=oT_sb[:, t * PT:(t + 1) * PT],
                                    identity=ident[:])
            for t in range(TPC):
                ob = small_p.tile([PT, H], F32, tag="ob")
                nc.vector.tensor_scalar_mul(ob, otp[:, t * PT:(t + 1) * PT],
                                            recip[:, t:t + 1])
                nc.gpsimd.dma_start(
                    out=outd[c * CH + t * PT: c * CH + (t + 1) * PT, :], in_=ob)

        # ---- interleaved main loop ----
        for c in range(3):
            project("k", c)
            project("v", c)
            project("q", c)
            oT = ps_outT.tile([H, CH], F32, tag="outT")
            sums = ps_sums.tile([1, CH], F32, tag="sums")
            attn_tiles(c, range((c + 1) * TPC), oT, sums)
            finalize(c, oT, sums)

        # chunk 3: Q first, off-diagonal attention, then K3/V3, diagonal tiles
        project("q", 3)
        oT = ps_outT.tile([H, CH], F32, tag="outT")
        sums = ps_sums.tile([1, CH], F32, tag="sums")
        attn_tiles(3, range(12), oT, sums)
        project("k", 3)
        project("v", 3)
        attn_tiles(3, range(12, 16), oT, sums)
        finalize(3, oT, sums)

    nc.compile()
    return nc


_NC_CACHE = {}


def _get_nc():
    if "nc" not in _NC_CACHE:
        _NC_CACHE["nc"] = build(S=2048, E=1024, H=128, n_cores=8)
    return _NC_CACHE["nc"]


def kernel(Q, K, V, mask=None, Wq=None, bq=None, Wk=None, bk=None,
           Wv=None, bv=None, **_):
    """Full-input entry point: Q/K/V [8, 2048, 1024] fp32 -> out [8, 2048, 128].

    Data-parallel over batch: core i computes batch element i. The causal
    mask input is ignored (causality is hardcoded in the kernel structure).
    """
    from concourse.bass_utils import run_bass_kernel_spmd

    B = Q.shape[0]
    nc = _get_nc()
    f32 = np.float32
    in_maps = []
    for i in range(B):
        in_maps.append({
            "Q": np.ascontiguousarray(Q[i], dtype=f32),
            "K": np.ascontiguousarray(K[i], dtype=f32),
            "V": np.ascontiguousarray(V[i], dtype=f32),
            "Wq": np.ascontiguousarray(Wq, dtype=f32),
            "Wk": np.ascontiguousarray(Wk, dtype=f32),
            "Wv": np.ascontiguousarray(Wv, dtype=f32),
            "bq": np.ascontiguousarray(bq, dtype=f32),
            "bk": np.ascontiguousarray(bk, dtype=f32),
            "bv": np.ascontiguousarray(bv, dtype=f32),
        })
    r = run_bass_kernel_spmd(nc, in_maps, list(range(B)))
    return np.stack([r.results[i]["out"] for i in range(B)]).astype(np.float32)


# revision 10
# speedup vs baseline: 18.7439x; 1.2516x over previous
"""Causal single-head attention kernel for TRN2 (one batch element per core).

Computes: out = softmax(causal((X_q Wq + bq)(X_k Wk + bk)^T / sqrt(H))) (X_v Wv + bv)
Shapes per core: Q,K,V [S, E]; Wq/Wk/Wv [E, H]; bq/bk/bv [H]; out [S, H].

v3 design:
- Input X transposes on the PE (bf16, via identity) hidden under the
  HBM-bound streaming phase; PSUM->SBUF copies split scalar/vector.
- DMA issue order: weights/biases first, then all 12 input-chunk DMAs in
  consumption order (Q3 before K3/V3), then constants -- so the first
  projection starts ~7us in and weights never queue behind bulk input.
- One interleaved loop: project K/V/Q of chunk c, then attention chunk c;
  next chunk's DMA + transposes overlap attention. Chunk 3 is split so its
  off-diagonal attention tiles run before K3/V3 are projected.
- Softmax denominators off the PE: gpsimd partition-reduce (axis=C) of each
  exp'd score tile, accumulated on vector; only a tiny [1,128]x[1,1]
  transpose-matmul per chunk stays on the PE.
- V tiles transposed via the DMA XBAR on the idle Sync queue (16 calls);
  output stores issued on gpsimd to keep Sync/Scalar queues unblocked.
"""

from contextlib import ExitStack

import numpy as np

import concourse.bacc as bacc
import concourse.bass as bass
import concourse.mybir as mybir
import concourse.tile as tile
from concourse.masks import make_identity

F32 = mybir.dt.float32
BF16 = mybir.dt.bfloat16

CH = 512          # Sq chunk width (psum bank)
PT = 128          # partition tile


def build(S=2048, E=1024, H=128, n_cores=8):
    """Build + compile the Bacc kernel. Returns nc."""
    EC = E // PT              # E chunks (8)
    NCHUNK = S // CH          # Sq chunks (4)
    TPC = CH // PT            # S-tiles per chunk (4)
    scale = float(H) ** -0.5

    nc = bacc.Bacc("TRN2", target_bir_lowering=False, debug=False,
                   num_devices=n_cores)

    Qd = nc.declare_dram_parameter("Q", [S, E], F32, isOutput=False)
    Kd = nc.declare_dram_parameter("K", [S, E], F32, isOutput=False)
    Vd = nc.declare_dram_parameter("V", [S, E], F32, isOutput=False)
    Wqd = nc.declare_dram_parameter("Wq", [E, H], F32, isOutput=False)
    Wkd = nc.declare_dram_parameter("Wk", [E, H], F32, isOutput=False)
    Wvd = nc.declare_dram_parameter("Wv", [E, H], F32, isOutput=False)
    bqd = nc.declare_dram_parameter("bq", [H], F32, isOutput=False)
    bkd = nc.declare_dram_parameter("bk", [H], F32, isOutput=False)
    bvd = nc.declare_dram_parameter("bv", [H], F32, isOutput=False)
    outd = nc.declare_dram_parameter("out", [S, H], F32, isOutput=True)

    xd = {"q": Qd, "k": Kd, "v": Vd}

    with tile.TileContext(nc) as tc, ExitStack() as ctx:
        persist = ctx.enter_context(tc.tile_pool(name="persist", bufs=1))
        xn_p = ctx.enter_context(tc.tile_pool(name="xn", bufs=1))
        xt_p = ctx.enter_context(tc.tile_pool(name="xt", bufs=12))
        ew_p = ctx.enter_context(tc.tile_pool(name="ew", bufs=8))
        small_p = ctx.enter_context(tc.tile_pool(name="small", bufs=6))

        ps_tp = ctx.enter_context(tc.tile_pool(name="ps_tp", bufs=2, space="PSUM"))
        ps_mm = ctx.enter_context(tc.tile_pool(name="ps_mm", bufs=3, space="PSUM"))
        ps_outT = ctx.enter_context(tc.tile_pool(name="ps_outT", bufs=1, space="PSUM"))
        ps_fin = ctx.enter_context(tc.tile_pool(name="ps_fin", bufs=1, space="PSUM"))
        ps_sums = ctx.enter_context(tc.tile_pool(name="ps_sums", bufs=1, space="PSUM"))

        # ---- weights + biases first (small, land before first projection) ----
        wts = {}
        for nm, d in (("k", Wkd), ("v", Wvd), ("q", Wqd)):
            w = persist.tile([PT, EC, H], BF16, tag=f"w{nm}")
            nc.gpsimd.dma_start(out=w, in_=d[:].rearrange("(c p) h -> p c h", p=PT))
            wts[nm] = w
        bias = {}
        for nm, d in (("k", bkd), ("v", bvd), ("q", bqd)):
            b = persist.tile([H, 1], F32, tag=f"b{nm}")
            nc.gpsimd.dma_start(out=b, in_=d[:].unsqueeze(1))
            bias[nm] = b

        # ---- input stream: all 12 chunk DMAs issued up front (gpsimd casts) ----
        stream = [("k", 0), ("v", 0), ("q", 0),
                  ("k", 1), ("v", 1), ("q", 1),
                  ("k", 2), ("v", 2), ("q", 2),
                  ("q", 3), ("k", 3), ("v", 3)]
        xn = {}

        def issue_xn(nm, c):
            t_ = xn_p.tile([PT, TPC, E], BF16, tag=f"xn_{nm}{c}",
                           name=f"xn_{nm}{c}")
            nc.gpsimd.dma_start(
                out=t_, in_=xd[nm][c * CH:(c + 1) * CH, :].rearrange(
                    "(t p) e -> p t e", p=PT))
            xn[(nm, c)] = t_

        for nm, c in stream[:3]:
            issue_xn(nm, c)

        # ---- constants (fit on gpsimd while chunk-0 inputs stream) ----
        ident = persist.tile([PT, PT], F32, tag="ident")
        make_identity(nc, ident)
        ident_b = persist.tile([PT, PT], BF16, tag="ident_b")
        make_identity(nc, ident_b)
        one_1 = persist.tile([1, 1], F32, tag="one_1")
        nc.gpsimd.memset(one_1, 1.0)
        ones_col = persist.tile([PT, 1], BF16, tag="ones_col")
        nc.gpsimd.memset(ones_col, 1.0)

        masks = []
        for m in range(TPC):
            mk = persist.tile([PT, CH], BF16, tag=f"mask{m}")
            nc.gpsimd.memset(mk, 1.0)
            # keep (=1.0) where f - p - 128*m >= 0 else fill 0.0
            nc.gpsimd.affine_select(
                out=mk, in_=mk, compare_op=mybir.AluOpType.is_ge,
                fill=0.0, base=-PT * m, pattern=[[1, CH]], channel_multiplier=-1,
            )
            masks.append(mk)

        for nm, c in stream[3:]:
            issue_xn(nm, c)

        # persistent projected tensors
        qT = [persist.tile([H, CH], BF16, tag=f"qT{c}", name=f"qT{c}")
              for c in range(NCHUNK)]
        kT = [persist.tile([H, CH], BF16, tag=f"kT{c}", name=f"kT{c}")
              for c in range(NCHUNK)]
        vnat = [persist.tile([PT, H], BF16, tag=f"v{j}", name=f"v{j}")
                for j in range(S // PT)]

        # ---- helpers ----
        def project(nm, c):
            """xn[(nm,c)] -> qT[c]/kT[c]/vnat[4c..4c+3]: PE transpose + PE mm."""
            xnt = xn[(nm, c)]
            w = wts[nm]
            b = bias[nm]
            xts = []
            for e in range(EC):
                tp = ps_tp.tile([PT, CH], BF16, tag="tp")
                for t in range(TPC):
                    nc.tensor.transpose(
                        out=tp[:, t * PT:(t + 1) * PT],
                        in_=xnt[:, t, e * PT:(e + 1) * PT],
                        identity=ident_b[:],
                    )
                xt = xt_p.tile([PT, CH], BF16, tag="xt")
                if e % 2 == 0:
                    nc.scalar.copy(out=xt, in_=tp)
                else:
                    nc.vector.tensor_copy(xt, tp)
                xts.append(xt)
            pj = ps_mm.tile([H, CH], F32, tag="mm")
            for e in range(EC):
                nc.tensor.matmul(pj, w[:, e, :], xts[e],
                                 start=(e == 0), stop=(e == EC - 1))
            if nm == "q":
                nc.vector.tensor_scalar_add(qT[c], pj, b[:])
            elif nm == "k":
                nc.vector.tensor_scalar_add(kT[c], pj, b[:])
            else:
                vTb = small_p.tile([H, CH], BF16, tag="vTb")
                nc.vector.tensor_scalar_add(vTb, pj, b[:])
                vtp = ps_tp.tile([PT, CH], BF16, tag="tp")
                for t in range(TPC):
                    nc.tensor.transpose(
                        out=vtp[:, t * PT:(t + 1) * PT],
                        in_=vTb[:, t * PT:(t + 1) * PT],
                        identity=ident_b[:],
                    )
                for t in range(TPC):
                    nc.vector.tensor_copy(vnat[c * TPC + t],
                                          vtp[:, t * PT:(t + 1) * PT])

        def attn_tiles(c, js, oT, sums):
            """Score/exp/PV for Sk tiles `js` of Sq chunk c.

            sums [1, CH] f32 PSUM accumulates softmax denominators via
            ones-vector matmuls."""
            nk = (c + 1) * TPC
            for j in js:
                wp = ps_mm.tile([PT, CH], F32, tag="mm")
                kc, kt = divmod(j, TPC)
                nc.tensor.matmul(wp, kT[kc][:, kt * PT:(kt + 1) * PT],
                                 qT[c], start=True, stop=True)
                ew = ew_p.tile([PT, CH], BF16, tag="ew")
                nc.scalar.activation(out=ew, in_=wp,
                                     func=mybir.ActivationFunctionType.Exp,
                                     scale=scale)
                m = j - c * TPC
                if m >= 0:
                    nc.vector.tensor_mul(ew, ew, masks[m])
                # PV accumulate on PE
                nc.tensor.matmul(oT, vnat[j][:], ew,
                                 start=(j == 0), stop=(j == nk - 1))
                # denominators on PE: ones^T @ ew accumulated in PSUM
                nc.tensor.matmul(sums, ones_col[:], ew,
                                 start=(j == 0), stop=(j == nk - 1))

        def finalize(c, oT, sums):
            # sums [1, CH] -> sumsT [128, TPC] (tiny PE matmuls), then recip
            sums_sb = small_p.tile([1, CH], F32, tag="sums_sb")
            nc.vector.tensor_copy(sums_sb, sums)
            sumsT = ps_mm.tile([PT, TPC], F32, tag="mm")
            for t in range(TPC):
                nc.tensor.matmul(sumsT[:, t:t + 1],
                                 sums_sb[0:1, t * PT:(t + 1) * PT],
                                 one_1[:], start=True, stop=True)
            recip = small_p.tile([PT, TPC], F32, tag="recip")
            nc.vector.reciprocal(recip, sumsT[:, 0:TPC])

            oT_sb = small_p.tile([H, CH], F32, tag="oT_sb")
            nc.scalar.copy(out=oT_sb, in_=oT)
            otp = ps_fin.tile([PT, CH], F32, tag="otp")
            for t in range(TPC):
                nc.tensor.transpose(out=otp[:, t * PT:(t + 1) * PT],
                                    in_=oT_sb[:, t * PT:(t + 1) * PT],
                                    identity=ident[:])
            for t in range(TPC):
                ob = small_p.tile([PT, H], F32, tag="ob")
                nc.vector.tensor_scalar_mul(ob, otp[:, t * PT:(t + 1) * PT],
                                            recip[:, t:t + 1])
                nc.gpsimd.dma_start(
                    out=outd[c * CH + t * PT: c * CH + (t + 1) * PT, :], in_=ob)

        # ---- interleaved main loop ----
        for c in range(3):
            project("k", c)
            project("v", c)
            project("q", c)
            oT = ps_outT.tile([H, CH], F32, tag="outT")
            sums = ps_sums.tile([1, CH], F32, tag="sums")
            attn_tiles(c, range((c + 1) * TPC), oT, sums)
            finalize(c, oT, sums)

        # chunk 3: Q first, off-diagonal attention, then K3/V3, diagonal tiles
        project("q", 3)
        oT = ps_outT.tile([H, CH], F32, tag="outT")
        sums = ps_sums.tile([1, CH], F32, tag="sums")
        attn_tiles(3, range(12), oT, sums)
        project("k", 3)
        project("v", 3)
        attn_tiles(3, range(12, 16), oT, sums)
        finalize(3, oT, sums)

    nc.compile()
    return nc


_NC_CACHE = {}


def _get_nc():
    if "nc" not in _NC_CACHE:
        _NC_CACHE["nc"] = build(S=2048, E=1024, H=128, n_cores=8)
    return _NC_CACHE["nc"]


def kernel(Q, K, V, mask=None, Wq=None, bq=None, Wk=None, bk=None,
           Wv=None, bv=None, **_):
    """Full-input entry point: Q/K/V [8, 2048, 1024] fp32 -> out [8, 2048, 128].

    Data-parallel over batch: core i computes batch element i. The causal
    mask input is ignored (causality is hardcoded in the kernel structure).
    """
    from concourse.bass_utils import run_bass_kernel_spmd

    B = Q.shape[0]
    nc = _get_nc()
    f32 = np.float32
    in_maps = []
    for i in range(B):
        in_maps.append({
            "Q": np.ascontiguousarray(Q[i], dtype=f32),
            "K": np.ascontiguousarray(K[i], dtype=f32),
            "V": np.ascontiguousarray(V[i], dtype=f32),
            "Wq": np.ascontiguousarray(Wq, dtype=f32),
            "Wk": np.ascontiguousarray(Wk, dtype=f32),
            "Wv": np.ascontiguousarray(Wv, dtype=f32),
            "bq": np.ascontiguousarray(bq, dtype=f32),
            "bk": np.ascontiguousarray(bk, dtype=f32),
            "bv": np.ascontiguousarray(bv, dtype=f32),
        })
    r = run_bass_kernel_spmd(nc, in_maps, list(range(B)))
    return np.stack([r.results[i]["out"] for i in range(B)]).astype(np.float32)
